# revision 20
# baseline (speedup 1.0000x reference)
"""Additive (Bahdanau) attention on 8 TRN2 NeuronCores, data-parallel over batch.

Reference math (per batch b):
  qh = queries @ W_q            [Q, H]
  kh = keys @ W_k               [K, H]
  scores[q,k] = sum_h w_v[h] * tanh(qh[q,h] + kh[k,h])
  scores[q,k] = -1e6 where k >= valid_len[b]
  out = softmax_k(scores) @ values

Device strategy (B=16 sharded 2 per core):
  - H=128 lives on the partition axis. khT [H, K] and qhT [H, Q] come from
    PE transposes of the natural loads followed by fp32 projection matmuls,
    evicted to bf16.
  - Per q: DVE tensor_scalar_add broadcasts qhT[:, q] over khT (bf16, split in
    K-halves so adds start before the full khT exists); per q-group one big
    ACT Tanh produces bf16 features (ACT is the roofline: B*Q*K*H/8 elems /
    128 lanes / 1.2GHz ~= 109us/core; the main loop runs tanh back-to-back).
  - Per (q, k-chunk): matmul lhsT=features[H,128] (stationary), rhs=w_v[H,1]
    -> scoresT column [128k, 1] into a one-bank PSUM tile laid out [128, KC*64].
  - Masking is fused into the Exp as a per-partition bias column built from a
    constant iota input and a broadcast valid_len: bias = (k_idx>=len)*-1e6.
    exp(score-1e6) underflows to exactly 0; scores are bounded (~|12|) so no
    max-subtraction is needed.
  - attnT @ [values | ones] accumulates [Q, 257]; the ones column gives the
    softmax denominator; one reciprocal + per-partition scale normalizes.
  - DMA: keys as two big half-DMAs (sync + scalar HWDGE queues), values via a
    single gpsimd SWDGE DMA that casts f32->bf16 in flight.
"""

import numpy as np

import concourse.bass as bass
import concourse.bacc as bacc
import concourse.mybir as mybir
import concourse.tile as tile
from concourse.bass_utils import run_bass_kernel_spmd

B, Q, K, D, H = 16, 64, 1024, 256, 128
NCORES = 8
BL = B // NCORES  # batches per core
KC = K // 128     # k-chunks of 128
DC = D // 128     # d-chunks of 128
QG = 8            # q-group size per Tanh instruction
NEG = -1.0e6

F32 = mybir.dt.float32
BF16 = mybir.dt.bfloat16
I32 = mybir.dt.int32
AF = mybir.ActivationFunctionType
ALU = mybir.AluOpType


def _emit(nc, tc, dram):
    queries, keys, values, vlens, cblobA, cblobB, out = dram
    QSCHED = [4, 4] + [8] * 6 + [4, 4]
    assert sum(QSCHED) == Q
    with (
        tc.tile_pool(name="const", bufs=1) as cpool,
        tc.tile_pool(name="io", bufs=2) as io,
        tc.tile_pool(name="work", bufs=2) as work,
        tc.tile_pool(name="sums", bufs=3) as sums_pool,
        tc.tile_pool(name="feat", bufs=3) as feat_pool,
        tc.tile_pool(name="psT", bufs=4, space=bass.MemorySpace.PSUM) as psT,
        tc.tile_pool(name="psP", bufs=2, space=bass.MemorySpace.PSUM) as psP,
        tc.tile_pool(name="psS", bufs=1, space=bass.MemorySpace.PSUM) as psS,
        tc.tile_pool(name="psO", bufs=1, space=bass.MemorySpace.PSUM) as psO,
    ):
        cbA = cpool.tile([128, 265], F32, tag="cbA")
        cbB = cpool.tile([128, 513], BF16, tag="cbB")
        nc.sync.dma_start(cbA[:], cblobA[:, :])
        nc.sync.dma_start(cbB[:], cblobB[:, :])
        ident_sb = cbA[:, 0:128]
        ones_sb = cbA[0:1, 128:256]
        kidx_sb = cbA[:, 256:264]
        wq_bf = cbB[:, 0:256]
        wk_bf = cbB[:, 256:512]
        wv_bf = cbB[:, 512:513]
        vl_i = cpool.tile([1, BL], I32, tag="vli")
        nc.sync.dma_start(vl_i[:], vlens[:, :])
        vl_f = cpool.tile([1, BL], F32, tag="vlf")
        nc.vector.tensor_copy(vl_f[:], vl_i[:])

        for b in range(BL):
            qnat = io.tile([Q, D], F32, tag="qnat")
            nc.sync.dma_start(qnat[:], queries[b, :, :])
            knat_all = io.tile([128, KC * D], F32, tag="knat")
            for kc in range(KC):
                nc.sync.dma_start(
                    knat_all[:, kc * D : (kc + 1) * D],
                    keys[b, kc * 128 : (kc + 1) * 128, :],
                )

            # ---- projections: khT [H, K] (half 0 first), qhT [H, Q] ----
            kTd = work.tile([128, DC * K], BF16, tag="kTd")
            khT = work.tile([128, K], F32, tag="khT")
            qT_sb = work.tile([128, DC * Q], BF16, tag="qT")
            qhT = work.tile([128, Q], F32, tag="qhT")

            def k_transposes(kc_list):
                for kc in kc_list:
                    for dc in range(DC):
                        tp = psT.tile([128, 128], F32, tag="tp")
                        nc.tensor.transpose(
                            tp[:],
                            knat_all[:, kc * D + dc * 128 : kc * D + (dc + 1) * 128],
                            ident_sb[:, :],
                        )
                        nc.vector.tensor_copy(
                            kTd[:, dc * K + kc * 128 : dc * K + (kc + 1) * 128], tp[:]
                        )

            def kh_half(nch):
                kh_ps = psP.tile([128, 512], F32, tag="proj")
                for dc in range(DC):
                    nc.tensor.matmul(
                        kh_ps[:],
                        wk_bf[:, dc * 128 : (dc + 1) * 128],
                        kTd[:, dc * K + nch * 512 : dc * K + nch * 512 + 512],
                        start=(dc == 0),
                        stop=(dc == DC - 1),
                    )
                nc.vector.tensor_copy(khT[:, nch * 512 : (nch + 1) * 512], kh_ps[:])

            k_transposes(range(0, KC // 2))
            kh_half(0)
            for dc in range(DC):
                tp = psT.tile([128, 128], F32, tag="tp")
                nc.tensor.transpose(
                    tp[:, 0:Q], qnat[:, dc * 128 : (dc + 1) * 128], ident_sb[0:Q, 0:Q]
                )
                nc.vector.tensor_copy(qT_sb[:, dc * Q : (dc + 1) * Q], tp[:, 0:Q])
            qh_ps = psP.tile([128, 512], F32, tag="proj")
            for dc in range(DC):
                nc.tensor.matmul(
                    qh_ps[:, 0:Q],
                    wq_bf[:, dc * 128 : (dc + 1) * 128],
                    qT_sb[:, dc * Q : (dc + 1) * Q],
                    start=(dc == 0),
                    stop=(dc == DC - 1),
                )
            nc.vector.tensor_copy(qhT[:], qh_ps[:, 0:Q])
            k_transposes(range(KC // 2, KC))
            kh_half(1)

            # ---- mask bias column: madd[p, kc] = (p + 128*kc >= len) * -1e6 ----
            ln_ps = psT.tile([128, 128], F32, tag="tp")
            nc.tensor.matmul(
                ln_ps[:, 0:1], ones_sb, vl_f[0:1, b : b + 1], start=True, stop=True
            )
            ln_col = work.tile([128, 1], F32, tag="lncol")
            nc.vector.tensor_copy(ln_col[:], ln_ps[:, 0:1])
            madd = work.tile([128, KC], F32, tag="madd")
            nc.vector.tensor_scalar(
                madd[:], kidx_sb, ln_col[:], NEG, op0=ALU.is_ge, op1=ALU.mult
            )

            # ---- features + scoresT ----
            scT_ps = psS.tile([128, 512], F32, tag="sc")
            q0 = 0
            for g, qg in enumerate(QSCHED):
                sums = sums_pool.tile([128, qg * K], F32, tag="sums")
                for j in range(qg):
                    q = q0 + j
                    nc.vector.tensor_scalar_add(
                        sums[:, j * K : (j + 1) * K], khT[:], qhT[:, q : q + 1]
                    )
                feat = feat_pool.tile([128, qg * K], BF16, tag="feat")
                nc.scalar.activation(feat[:], sums[:], AF.Tanh)
                for j in range(qg):
                    q = q0 + j
                    for kc in range(KC):
                        nc.tensor.matmul(
                            scT_ps[:, kc * 64 + q : kc * 64 + q + 1],
                            feat[:, j * K + kc * 128 : j * K + (kc + 1) * 128],
                            wv_bf,
                            start=True,
                            stop=True,
                        )
                q0 += qg

            # ---- masked exp (bias fuses the mask) ----
            pT = work.tile([128, 512], BF16, tag="pT")
            for kc in range(KC):
                nc.scalar.activation(
                    pT[:, kc * 64 : (kc + 1) * 64],
                    scT_ps[:, kc * 64 : (kc + 1) * 64],
                    AF.Exp,
                    bias=madd[:, kc : kc + 1],
                )

            # ---- values (cast to bf16 in the SWDGE DMA) with ones columns ----
            vaug = work.tile([128, KC * 260], BF16, tag="vaug")
            for kc in range(KC):
                nc.gpsimd.dma_start(
                    vaug[:, kc * 260 : kc * 260 + 256],
                    values[b, kc * 128 : (kc + 1) * 128, :],
                )
                nc.gpsimd.memset(vaug[:, kc * 260 + 256 : kc * 260 + 257], 1.0)

            # ---- attnT @ [values | ones], normalize, store ----
            oaug_ps = psO.tile([Q, 257], F32, tag="oa")
            for kc in range(KC):
                nc.tensor.matmul(
                    oaug_ps[:],
                    pT[:, kc * 64 : (kc + 1) * 64],
                    vaug[:, kc * 260 : kc * 260 + 257],
                    start=(kc == 0),
                    stop=(kc == KC - 1),
                )
            recip = work.tile([Q, 1], F32, tag="recip")
            nc.vector.reciprocal(recip[:], oaug_ps[:, 256:257])
            out_sb = work.tile([Q, D], F32, tag="osb")
            nc.vector.tensor_scalar_mul(out_sb[:], oaug_ps[:, 0:256], recip[:])
            nc.sync.dma_start(out[b, :, :], out_sb[:])


def build():
    nc = bacc.Bacc("TRN2", target_bir_lowering=False, debug=False, num_devices=NCORES)
    dram = (
        nc.declare_dram_parameter("queries", [BL, Q, D], F32, isOutput=False),
        nc.declare_dram_parameter("keys", [BL, K, D], F32, isOutput=False),
        nc.declare_dram_parameter("values", [BL, K, D], F32, isOutput=False),
        nc.declare_dram_parameter("valid_lens", [1, BL], I32, isOutput=False),
        nc.declare_dram_parameter("cblobA", [128, 265], F32, isOutput=False),
        nc.declare_dram_parameter("cblobB", [128, 513], BF16, isOutput=False),
        nc.declare_dram_parameter("out", [BL, Q, D], F32, isOutput=True),
    )
    with tile.TileContext(nc) as tc:
        _emit(nc, tc, dram)
    nc.compile()
    return nc


_NC = None


def make_in_maps(queries, keys, values, valid_lens, W_q, W_k, w_v):
    queries = np.ascontiguousarray(np.asarray(queries, dtype=np.float32))
    keys = np.ascontiguousarray(np.asarray(keys, dtype=np.float32))
    values = np.ascontiguousarray(np.asarray(values, dtype=np.float32))
    valid_lens = np.asarray(valid_lens, dtype=np.int32)
    W_q = np.asarray(W_q, dtype=np.float32)
    W_k = np.asarray(W_k, dtype=np.float32)
    w_v = np.asarray(w_v, dtype=np.float32).reshape(H)
    cblobA = np.zeros((128, 265), dtype=np.float32)
    cblobA[:, 0:128] = np.eye(128, dtype=np.float32)
    cblobA[0, 128:256] = 1.0
    cblobA[:, 256:264] = (
        np.arange(128, dtype=np.float32)[:, None]
        + 128.0 * np.arange(KC, dtype=np.float32)[None, :]
    )
    cblobA[:, 264] = w_v
    import ml_dtypes
    cblobB = np.zeros((128, 513), dtype=ml_dtypes.bfloat16)
    cblobB[:, 0:128] = W_q[0:128, :].astype(ml_dtypes.bfloat16)
    cblobB[:, 128:256] = W_q[128:256, :].astype(ml_dtypes.bfloat16)
    cblobB[:, 256:384] = W_k[0:128, :].astype(ml_dtypes.bfloat16)
    cblobB[:, 384:512] = W_k[128:256, :].astype(ml_dtypes.bfloat16)
    cblobB[:, 512] = w_v.astype(ml_dtypes.bfloat16)
    in_maps = []
    for i in range(NCORES):
        s = slice(i * BL, (i + 1) * BL)
        in_maps.append(
            {
                "queries": np.ascontiguousarray(queries[s]),
                "keys": np.ascontiguousarray(keys[s]),
                "values": np.ascontiguousarray(values[s]),
                "valid_lens": np.ascontiguousarray(valid_lens[s].reshape(1, BL)),
                "cblobA": cblobA,
                "cblobB": cblobB,
            }
        )
    return in_maps


def kernel(queries, keys, values, valid_lens, W_q, W_k, w_v):
    global _NC
    if _NC is None:
        _NC = build()
    in_maps = make_in_maps(queries, keys, values, valid_lens, W_q, W_k, w_v)
    res = run_bass_kernel_spmd(_NC, in_maps, core_ids=list(range(NCORES)))
    return np.concatenate([res.results[i]["out"] for i in range(NCORES)], axis=0)


# revision 21
# speedup vs baseline: 1.0161x; 1.0161x over previous
"""Additive (Bahdanau) attention on 8 TRN2 NeuronCores, data-parallel over batch.

Reference math (per batch b):
  qh = queries @ W_q            [Q, H]
  kh = keys @ W_k               [K, H]
  scores[q,k] = sum_h w_v[h] * tanh(qh[q,h] + kh[k,h])
  scores[q,k] = -1e6 where k >= valid_len[b]
  out = softmax_k(scores) @ values

Device strategy (B=16 sharded 2 per core):
  - H=128 lives on the partition axis. khT [H, K] and qhT [H, Q] come from
    PE transposes of the natural loads followed by fp32 projection matmuls,
    evicted to bf16.
  - Per q: DVE tensor_scalar_add broadcasts qhT[:, q] over khT (bf16, split in
    K-halves so adds start before the full khT exists); per q-group one big
    ACT Tanh produces bf16 features (ACT is the roofline: B*Q*K*H/8 elems /
    128 lanes / 1.2GHz ~= 109us/core; the main loop runs tanh back-to-back).
  - Per (q, k-chunk): matmul lhsT=features[H,128] (stationary), rhs=w_v[H,1]
    -> scoresT column [128k, 1] into a one-bank PSUM tile laid out [128, KC*64].
  - Masking is fused into the Exp as a per-partition bias column built from a
    constant iota input and a broadcast valid_len: bias = (k_idx>=len)*-1e6.
    exp(score-1e6) underflows to exactly 0; scores are bounded (~|12|) so no
    max-subtraction is needed.
  - attnT @ [values | ones] accumulates [Q, 257]; the ones column gives the
    softmax denominator; one reciprocal + per-partition scale normalizes.
  - DMA: keys as two big half-DMAs (sync + scalar HWDGE queues), values via a
    single gpsimd SWDGE DMA that casts f32->bf16 in flight.
"""

import numpy as np

import concourse.bass as bass
import concourse.bacc as bacc
import concourse.mybir as mybir
import concourse.tile as tile
from concourse.bass_utils import run_bass_kernel_spmd

B, Q, K, D, H = 16, 64, 1024, 256, 128
NCORES = 8
BL = B // NCORES  # batches per core
KC = K // 128     # k-chunks of 128
DC = D // 128     # d-chunks of 128
QG = 8            # q-group size per Tanh instruction
NEG = -1.0e6

F32 = mybir.dt.float32
BF16 = mybir.dt.bfloat16
I32 = mybir.dt.int32
AF = mybir.ActivationFunctionType
ALU = mybir.AluOpType


def _emit(nc, tc, dram):
    queries, keys, values, vlens, cblobA, cblobB, out = dram
    QSCHED = [4, 4] + [8] * 6 + [4, 4]
    assert sum(QSCHED) == Q
    with (
        tc.tile_pool(name="const", bufs=1) as cpool,
        tc.tile_pool(name="io", bufs=2) as io,
        tc.tile_pool(name="work", bufs=2) as work,
        tc.tile_pool(name="sums", bufs=3) as sums_pool,
        tc.tile_pool(name="feat", bufs=3) as feat_pool,
        tc.tile_pool(name="psT", bufs=3, space=bass.MemorySpace.PSUM) as psT,
        tc.tile_pool(name="psP", bufs=2, space=bass.MemorySpace.PSUM) as psP,
        tc.tile_pool(name="psS", bufs=2, space=bass.MemorySpace.PSUM) as psS,
        tc.tile_pool(name="psO", bufs=1, space=bass.MemorySpace.PSUM) as psO,
    ):
        cbA = cpool.tile([128, 265], F32, tag="cbA")
        cbB = cpool.tile([128, 513], BF16, tag="cbB")
        nc.sync.dma_start(cbA[:], cblobA[:, :])
        nc.sync.dma_start(cbB[:], cblobB[:, :])
        ident_sb = cbA[:, 0:128]
        ones_sb = cbA[0:1, 128:256]
        kidx_sb = cbA[:, 256:264]
        wq_bf = cbB[:, 0:256]
        wk_bf = cbB[:, 256:512]
        wv_bf = cbB[:, 512:513]
        vl_i = cpool.tile([1, BL], I32, tag="vli")
        nc.sync.dma_start(vl_i[:], vlens[:, :])
        vl_f = cpool.tile([1, BL], F32, tag="vlf")
        nc.vector.tensor_copy(vl_f[:], vl_i[:])

        for b in range(BL):
            qnat = io.tile([Q, D], F32, tag="qnat")
            nc.sync.dma_start(qnat[:], queries[b, :, :])
            knat_all = io.tile([128, KC * D], F32, tag="knat")
            for kc in range(KC):
                nc.sync.dma_start(
                    knat_all[:, kc * D : (kc + 1) * D],
                    keys[b, kc * 128 : (kc + 1) * 128, :],
                )

            # ---- projections: khT [H, K] (half 0 first), qhT [H, Q] ----
            kTd = work.tile([128, DC * K], BF16, tag="kTd")
            khT = work.tile([128, K], F32, tag="khT")
            qT_sb = work.tile([128, DC * Q], BF16, tag="qT")
            qhT = work.tile([128, Q], F32, tag="qhT")

            def k_transposes(kc_list):
                for kc in kc_list:
                    for dc in range(DC):
                        tp = psT.tile([128, 128], F32, tag="tp")
                        nc.tensor.transpose(
                            tp[:],
                            knat_all[:, kc * D + dc * 128 : kc * D + (dc + 1) * 128],
                            ident_sb[:, :],
                        )
                        nc.vector.tensor_copy(
                            kTd[:, dc * K + kc * 128 : dc * K + (kc + 1) * 128], tp[:]
                        )

            def kh_half(nch):
                kh_ps = psP.tile([128, 512], F32, tag="proj")
                for dc in range(DC):
                    nc.tensor.matmul(
                        kh_ps[:],
                        wk_bf[:, dc * 128 : (dc + 1) * 128],
                        kTd[:, dc * K + nch * 512 : dc * K + nch * 512 + 512],
                        start=(dc == 0),
                        stop=(dc == DC - 1),
                    )
                nc.vector.tensor_copy(khT[:, nch * 512 : (nch + 1) * 512], kh_ps[:])

            k_transposes(range(0, KC // 2))
            kh_half(0)
            for dc in range(DC):
                tp = psT.tile([128, 128], F32, tag="tp")
                nc.tensor.transpose(
                    tp[:, 0:Q], qnat[:, dc * 128 : (dc + 1) * 128], ident_sb[0:Q, 0:Q]
                )
                nc.vector.tensor_copy(qT_sb[:, dc * Q : (dc + 1) * Q], tp[:, 0:Q])
            qh_ps = psP.tile([128, 512], F32, tag="proj")
            for dc in range(DC):
                nc.tensor.matmul(
                    qh_ps[:, 0:Q],
                    wq_bf[:, dc * 128 : (dc + 1) * 128],
                    qT_sb[:, dc * Q : (dc + 1) * Q],
                    start=(dc == 0),
                    stop=(dc == DC - 1),
                )
            nc.vector.tensor_copy(qhT[:], qh_ps[:, 0:Q])
            k_transposes(range(KC // 2, KC))
            kh_half(1)

            # ---- mask bias column: madd[p, kc] = (p + 128*kc >= len) * -1e6 ----
            ln_ps = psT.tile([128, 128], F32, tag="tp")
            nc.tensor.matmul(
                ln_ps[:, 0:1], ones_sb, vl_f[0:1, b : b + 1], start=True, stop=True
            )
            ln_col = work.tile([128, 1], F32, tag="lncol")
            nc.vector.tensor_copy(ln_col[:], ln_ps[:, 0:1])
            madd = work.tile([128, KC], F32, tag="madd")
            nc.vector.tensor_scalar(
                madd[:], kidx_sb, ln_col[:], NEG, op0=ALU.is_ge, op1=ALU.mult
            )

            # ---- features + scoresT ----
            scT_ps = psS.tile([128, 512], F32, tag="sc")
            q0 = 0
            for g, qg in enumerate(QSCHED):
                sums = sums_pool.tile([128, qg * K], F32, tag="sums")
                for j in range(qg):
                    q = q0 + j
                    nc.vector.tensor_scalar_add(
                        sums[:, j * K : (j + 1) * K], khT[:], qhT[:, q : q + 1]
                    )
                feat = feat_pool.tile([128, qg * K], BF16, tag="feat")
                nc.scalar.activation(feat[:], sums[:], AF.Tanh)
                for j in range(qg):
                    q = q0 + j
                    for kc in range(KC):
                        nc.tensor.matmul(
                            scT_ps[:, kc * 64 + q : kc * 64 + q + 1],
                            feat[:, j * K + kc * 128 : j * K + (kc + 1) * 128],
                            wv_bf,
                            start=True,
                            stop=True,
                        )
                q0 += qg

            # ---- masked exp (bias fuses the mask) ----
            pT = work.tile([128, 512], BF16, tag="pT")
            for kc in range(KC):
                nc.scalar.activation(
                    pT[:, kc * 64 : (kc + 1) * 64],
                    scT_ps[:, kc * 64 : (kc + 1) * 64],
                    AF.Exp,
                    bias=madd[:, kc : kc + 1],
                )

            # ---- values (cast to bf16 in the SWDGE DMA) with ones columns ----
            vaug = work.tile([128, KC * 260], BF16, tag="vaug")
            for kc in range(KC):
                nc.gpsimd.dma_start(
                    vaug[:, kc * 260 : kc * 260 + 256],
                    values[b, kc * 128 : (kc + 1) * 128, :],
                )
                nc.gpsimd.memset(vaug[:, kc * 260 + 256 : kc * 260 + 257], 1.0)

            # ---- attnT @ [values | ones], normalize, store ----
            oaug_ps = psO.tile([Q, 257], F32, tag="oa")
            for kc in range(KC):
                nc.tensor.matmul(
                    oaug_ps[:],
                    pT[:, kc * 64 : (kc + 1) * 64],
                    vaug[:, kc * 260 : kc * 260 + 257],
                    start=(kc == 0),
                    stop=(kc == KC - 1),
                )
            recip = work.tile([Q, 1], F32, tag="recip")
            nc.vector.reciprocal(recip[:], oaug_ps[:, 256:257])
            out_sb = work.tile([Q, D], F32, tag="osb")
            nc.vector.tensor_scalar_mul(out_sb[:], oaug_ps[:, 0:256], recip[:])
            nc.sync.dma_start(out[b, :, :], out_sb[:])


def build():
    nc = bacc.Bacc("TRN2", target_bir_lowering=False, debug=False, num_devices=NCORES)
    dram = (
        nc.declare_dram_parameter("queries", [BL, Q, D], F32, isOutput=False),
        nc.declare_dram_parameter("keys", [BL, K, D], F32, isOutput=False),
        nc.declare_dram_parameter("values", [BL, K, D], F32, isOutput=False),
        nc.declare_dram_parameter("valid_lens", [1, BL], I32, isOutput=False),
        nc.declare_dram_parameter("cblobA", [128, 265], F32, isOutput=False),
        nc.declare_dram_parameter("cblobB", [128, 513], BF16, isOutput=False),
        nc.declare_dram_parameter("out", [BL, Q, D], F32, isOutput=True),
    )
    with tile.TileContext(nc) as tc:
        _emit(nc, tc, dram)
    nc.compile()
    return nc


_NC = None


def make_in_maps(queries, keys, values, valid_lens, W_q, W_k, w_v):
    queries = np.ascontiguousarray(np.asarray(queries, dtype=np.float32))
    keys = np.ascontiguousarray(np.asarray(keys, dtype=np.float32))
    values = np.ascontiguousarray(np.asarray(values, dtype=np.float32))
    valid_lens = np.asarray(valid_lens, dtype=np.int32)
    W_q = np.asarray(W_q, dtype=np.float32)
    W_k = np.asarray(W_k, dtype=np.float32)
    w_v = np.asarray(w_v, dtype=np.float32).reshape(H)
    cblobA = np.zeros((128, 265), dtype=np.float32)
    cblobA[:, 0:128] = np.eye(128, dtype=np.float32)
    cblobA[0, 128:256] = 1.0
    cblobA[:, 256:264] = (
        np.arange(128, dtype=np.float32)[:, None]
        + 128.0 * np.arange(KC, dtype=np.float32)[None, :]
    )
    cblobA[:, 264] = w_v
    import ml_dtypes
    cblobB = np.zeros((128, 513), dtype=ml_dtypes.bfloat16)
    cblobB[:, 0:128] = W_q[0:128, :].astype(ml_dtypes.bfloat16)
    cblobB[:, 128:256] = W_q[128:256, :].astype(ml_dtypes.bfloat16)
    cblobB[:, 256:384] = W_k[0:128, :].astype(ml_dtypes.bfloat16)
    cblobB[:, 384:512] = W_k[128:256, :].astype(ml_dtypes.bfloat16)
    cblobB[:, 512] = w_v.astype(ml_dtypes.bfloat16)
    in_maps = []
    for i in range(NCORES):
        s = slice(i * BL, (i + 1) * BL)
        in_maps.append(
            {
                "queries": np.ascontiguousarray(queries[s]),
                "keys": np.ascontiguousarray(keys[s]),
                "values": np.ascontiguousarray(values[s]),
                "valid_lens": np.ascontiguousarray(valid_lens[s].reshape(1, BL)),
                "cblobA": cblobA,
                "cblobB": cblobB,
            }
        )
    return in_maps


def kernel(queries, keys, values, valid_lens, W_q, W_k, w_v):
    global _NC
    if _NC is None:
        _NC = build()
    in_maps = make_in_maps(queries, keys, values, valid_lens, W_q, W_k, w_v)
    res = run_bass_kernel_spmd(_NC, in_maps, core_ids=list(range(NCORES)))
    return np.concatenate([res.results[i]["out"] for i in range(NCORES)], axis=0)


# revision 34
# speedup vs baseline: 1.0421x; 1.0256x over previous
"""Additive (Bahdanau) attention on 8 TRN2 NeuronCores, data-parallel over batch.

Reference math (per batch b):
  qh = queries @ W_q            [Q, H]
  kh = keys @ W_k               [K, H]
  scores[q,k] = sum_h w_v[h] * tanh(qh[q,h] + kh[k,h])
  scores[q,k] = -1e6 where k >= valid_len[b]
  out = softmax_k(scores) @ values

Shapes: B=16, Q=64, K=1024, D=256, H=128. B is sharded 2 per core; no
collectives. The roofline is the ScalarE (ACT) tanh pass over B*Q*K*H
elements: 16.8M/core / 128 lanes / 1.2GHz ~= 110us; the kernel runs the
16 tanh instructions back-to-back and hides everything else under them
(measured ~120us ACT busy, ~145us total incl. ~8.5us NEFF startup and
~10us Tile drain barrier).

Device strategy per core (2 batches):
  - H=128 on the partition axis. keys/queries are PE-transposed (identity
    matmul) and projected with bf16 weights (pre-packed on host into one
    bf16 blob): khT [H, K] f32, qhT [H, Q] f32.
  - Per q: DVE tensor_scalar_add broadcasts qhT[:, q] over khT (fp32 2x
    mode, ~746ns); per q-group one big ACT Tanh -> bf16 features. Group
    schedule [4,4,8*6,4,2,2] shortens the first-tanh latency and the tail.
    Batch 0's first group fuses the add into ACT's per-partition bias
    (tanh(khT + qhT[:,q])) so the first tanh needs no DVE adds at all.
  - Per (q, k-chunk): matmul lhsT=features[H,128] (stationary), rhs=w_v
    [H,1] -> scoresT column [128k, 1] into a one-bank PSUM tile [128, 8*64].
  - Masking fuses into the Exp bias: madd[p,kc] = (p+128*kc >= len)*-1e6
    built from a constant iota input and a ones-matmul broadcast of
    valid_len. exp(score-1e6) underflows to exactly 0; scores are bounded
    (|s| <~ 12) so no max-subtraction is needed (softmax is shift-invariant).
  - attnT @ [values | ones] accumulates [Q, 257] over k-chunks; the ones
    column is the softmax denominator; one reciprocal + per-partition
    scale normalizes. values are cast f32->bf16 inside gpsimd SWDGE DMAs.
  - DMA: small-constants blob first, keys split across the sync HWDGE and
    gpsimd SWDGE queues, weights blob + valid_lens behind the first keys
    half, output on sync.
"""

import numpy as np

import concourse.bass as bass
import concourse.bacc as bacc
import concourse.mybir as mybir
import concourse.tile as tile
from concourse.bass_utils import run_bass_kernel_spmd

B, Q, K, D, H = 16, 64, 1024, 256, 128
NCORES = 8
BL = B // NCORES  # batches per core
KC = K // 128     # k-chunks of 128
DC = D // 128     # d-chunks of 128
QG = 8            # q-group size per Tanh instruction
NEG = -1.0e6

F32 = mybir.dt.float32
BF16 = mybir.dt.bfloat16
I32 = mybir.dt.int32
AF = mybir.ActivationFunctionType
ALU = mybir.AluOpType


def _emit(nc, tc, dram):
    queries, keys, values, vlens, cblobA, cblobB, out = dram
    QSCHED = [4, 4] + [8] * 6 + [4, 2, 2]
    assert sum(QSCHED) == Q
    with (
        tc.tile_pool(name="const", bufs=1) as cpool,
        tc.tile_pool(name="io", bufs=2) as io,
        tc.tile_pool(name="work", bufs=2) as work,
        tc.tile_pool(name="sums", bufs=3) as sums_pool,
        tc.tile_pool(name="feat", bufs=3) as feat_pool,
        tc.tile_pool(name="psT", bufs=3, space=bass.MemorySpace.PSUM) as psT,
        tc.tile_pool(name="psP", bufs=2, space=bass.MemorySpace.PSUM) as psP,
        tc.tile_pool(name="psS", bufs=2, space=bass.MemorySpace.PSUM) as psS,
        tc.tile_pool(name="psO", bufs=1, space=bass.MemorySpace.PSUM) as psO,
    ):
        cbA = cpool.tile([128, 265], F32, tag="cbA")
        cbB = cpool.tile([128, 513], BF16, tag="cbB")
        nc.sync.dma_start(cbA[:], cblobA[:, :])
        ident_sb = cbA[:, 0:128]
        ones_sb = cbA[0:1, 128:256]
        kidx_sb = cbA[:, 256:264]
        wq_bf = cbB[:, 0:256]
        wk_bf = cbB[:, 256:512]
        wv_bf = cbB[:, 512:513]
        vl_i = cpool.tile([1, BL], I32, tag="vli")
        vl_f = cpool.tile([1, BL], F32, tag="vlf")

        for b in range(BL):
            knat_all = io.tile([128, KC * D], F32, tag="knat")
            for kc in range(KC // 2):
                nc.sync.dma_start(
                    knat_all[:, kc * D : (kc + 1) * D],
                    keys[b, kc * 128 : (kc + 1) * 128, :],
                )
            for kc in range(KC // 2, KC):
                nc.gpsimd.dma_start(
                    knat_all[:, kc * D : (kc + 1) * D],
                    keys[b, kc * 128 : (kc + 1) * 128, :],
                )
            qnat = io.tile([Q, D], F32, tag="qnat")
            nc.sync.dma_start(qnat[:], queries[b, :, :])
            if b == 0:
                nc.sync.dma_start(cbB[:], cblobB[:, :])
                nc.sync.dma_start(vl_i[:], vlens[:, :])

            # ---- projections: khT [H, K] (half 0 first), qhT [H, Q] ----
            kTd = work.tile([128, DC * K], BF16, tag="kTd")
            khT = work.tile([128, K], F32, tag="khT")
            qT_sb = work.tile([128, DC * Q], BF16, tag="qT")
            qhT = work.tile([128, Q], F32, tag="qhT")

            def k_transposes(kc_list):
                for kc in kc_list:
                    for dc in range(DC):
                        tp = psT.tile([128, 128], F32, tag="tp")
                        nc.tensor.transpose(
                            tp[:],
                            knat_all[:, kc * D + dc * 128 : kc * D + (dc + 1) * 128],
                            ident_sb[:, :],
                        )
                        nc.vector.tensor_copy(
                            kTd[:, dc * K + kc * 128 : dc * K + (kc + 1) * 128], tp[:]
                        )

            def kh_half(nch):
                kh_ps = psP.tile([128, 512], F32, tag="proj")
                for dc in range(DC):
                    nc.tensor.matmul(
                        kh_ps[:],
                        wk_bf[:, dc * 128 : (dc + 1) * 128],
                        kTd[:, dc * K + nch * 512 : dc * K + nch * 512 + 512],
                        start=(dc == 0),
                        stop=(dc == DC - 1),
                    )
                nc.vector.tensor_copy(khT[:, nch * 512 : (nch + 1) * 512], kh_ps[:])

            k_transposes(range(0, KC // 2))
            kh_half(0)
            for dc in range(DC):
                tp = psT.tile([128, 128], F32, tag="tp")
                nc.tensor.transpose(
                    tp[:, 0:Q], qnat[:, dc * 128 : (dc + 1) * 128], ident_sb[0:Q, 0:Q]
                )
                nc.vector.tensor_copy(qT_sb[:, dc * Q : (dc + 1) * Q], tp[:, 0:Q])
            qh_ps = psP.tile([128, 512], F32, tag="proj")
            for dc in range(DC):
                nc.tensor.matmul(
                    qh_ps[:, 0:Q],
                    wq_bf[:, dc * 128 : (dc + 1) * 128],
                    qT_sb[:, dc * Q : (dc + 1) * Q],
                    start=(dc == 0),
                    stop=(dc == DC - 1),
                )
            nc.vector.tensor_copy(qhT[:], qh_ps[:, 0:Q])
            k_transposes(range(KC // 2, KC))
            kh_half(1)

            # ---- mask bias column: madd[p, kc] = (p + 128*kc >= len) * -1e6 ----
            if b == 0:
                nc.vector.tensor_copy(vl_f[:], vl_i[:])
            ln_ps = psT.tile([128, 128], F32, tag="tp")
            nc.tensor.matmul(
                ln_ps[:, 0:1], ones_sb, vl_f[0:1, b : b + 1], start=True, stop=True
            )
            ln_col = work.tile([128, 1], F32, tag="lncol")
            nc.vector.tensor_copy(ln_col[:], ln_ps[:, 0:1])
            madd = work.tile([128, KC], F32, tag="madd")
            nc.vector.tensor_scalar(
                madd[:], kidx_sb, ln_col[:], NEG, op0=ALU.is_ge, op1=ALU.mult
            )

            # ---- features + scoresT ----
            scT_ps = psS.tile([128, 512], F32, tag="sc")
            q0 = 0
            for g, qg in enumerate(QSCHED):
                feat = feat_pool.tile([128, qg * K], BF16, tag="feat")
                if b == 0 and g == 0:
                    for j in range(qg):
                        q = q0 + j
                        nc.scalar.activation(
                            feat[:, j * K : (j + 1) * K],
                            khT[:],
                            AF.Tanh,
                            bias=qhT[:, q : q + 1],
                        )
                else:
                    sums = sums_pool.tile([128, qg * K], F32, tag="sums")
                    for j in range(qg):
                        q = q0 + j
                        nc.vector.tensor_scalar_add(
                            sums[:, j * K : (j + 1) * K], khT[:], qhT[:, q : q + 1]
                        )
                    nc.scalar.activation(feat[:], sums[:], AF.Tanh)
                for j in range(qg):
                    q = q0 + j
                    for kc in range(KC):
                        nc.tensor.matmul(
                            scT_ps[:, kc * 64 + q : kc * 64 + q + 1],
                            feat[:, j * K + kc * 128 : j * K + (kc + 1) * 128],
                            wv_bf,
                            start=True,
                            stop=True,
                        )
                q0 += qg

            # ---- masked exp (bias fuses the mask) ----
            pT = work.tile([128, 512], BF16, tag="pT")
            for kc in range(KC):
                nc.scalar.activation(
                    pT[:, kc * 64 : (kc + 1) * 64],
                    scT_ps[:, kc * 64 : (kc + 1) * 64],
                    AF.Exp,
                    bias=madd[:, kc : kc + 1],
                )

            # ---- values (cast to bf16 in the SWDGE DMA) with ones columns ----
            vaug = work.tile([128, KC * 260], BF16, tag="vaug")
            for kc in range(KC):
                nc.gpsimd.dma_start(
                    vaug[:, kc * 260 : kc * 260 + 256],
                    values[b, kc * 128 : (kc + 1) * 128, :],
                )
                nc.gpsimd.memset(vaug[:, kc * 260 + 256 : kc * 260 + 257], 1.0)

            # ---- attnT @ [values | ones], normalize, store ----
            oaug_ps = psO.tile([Q, 257], F32, tag="oa")
            for kc in range(KC):
                nc.tensor.matmul(
                    oaug_ps[:],
                    pT[:, kc * 64 : (kc + 1) * 64],
                    vaug[:, kc * 260 : kc * 260 + 257],
                    start=(kc == 0),
                    stop=(kc == KC - 1),
                )
            recip = work.tile([Q, 1], F32, tag="recip")
            nc.vector.reciprocal(recip[:], oaug_ps[:, 256:257])
            out_sb = work.tile([Q, D], F32, tag="osb")
            nc.vector.tensor_scalar_mul(out_sb[:], oaug_ps[:, 0:256], recip[:])
            nc.sync.dma_start(out[b, :, :], out_sb[:])


def build():
    nc = bacc.Bacc("TRN2", target_bir_lowering=False, debug=False, num_devices=NCORES)
    dram = (
        nc.declare_dram_parameter("queries", [BL, Q, D], F32, isOutput=False),
        nc.declare_dram_parameter("keys", [BL, K, D], F32, isOutput=False),
        nc.declare_dram_parameter("values", [BL, K, D], F32, isOutput=False),
        nc.declare_dram_parameter("valid_lens", [1, BL], I32, isOutput=False),
        nc.declare_dram_parameter("cblobA", [128, 265], F32, isOutput=False),
        nc.declare_dram_parameter("cblobB", [128, 513], BF16, isOutput=False),
        nc.declare_dram_parameter("out", [BL, Q, D], F32, isOutput=True),
    )
    with tile.TileContext(nc) as tc:
        _emit(nc, tc, dram)
    nc.compile()
    return nc


_NC = None


def make_in_maps(queries, keys, values, valid_lens, W_q, W_k, w_v):
    queries = np.ascontiguousarray(np.asarray(queries, dtype=np.float32))
    keys = np.ascontiguousarray(np.asarray(keys, dtype=np.float32))
    values = np.ascontiguousarray(np.asarray(values, dtype=np.float32))
    valid_lens = np.asarray(valid_lens, dtype=np.int32)
    W_q = np.asarray(W_q, dtype=np.float32)
    W_k = np.asarray(W_k, dtype=np.float32)
    w_v = np.asarray(w_v, dtype=np.float32).reshape(H)
    cblobA = np.zeros((128, 265), dtype=np.float32)
    cblobA[:, 0:128] = np.eye(128, dtype=np.float32)
    cblobA[0, 128:256] = 1.0
    cblobA[:, 256:264] = (
        np.arange(128, dtype=np.float32)[:, None]
        + 128.0 * np.arange(KC, dtype=np.float32)[None, :]
    )
    cblobA[:, 264] = w_v
    import ml_dtypes
    cblobB = np.zeros((128, 513), dtype=ml_dtypes.bfloat16)
    cblobB[:, 0:128] = W_q[0:128, :].astype(ml_dtypes.bfloat16)
    cblobB[:, 128:256] = W_q[128:256, :].astype(ml_dtypes.bfloat16)
    cblobB[:, 256:384] = W_k[0:128, :].astype(ml_dtypes.bfloat16)
    cblobB[:, 384:512] = W_k[128:256, :].astype(ml_dtypes.bfloat16)
    cblobB[:, 512] = w_v.astype(ml_dtypes.bfloat16)
    in_maps = []
    for i in range(NCORES):
        s = slice(i * BL, (i + 1) * BL)
        in_maps.append(
            {
                "queries": np.ascontiguousarray(queries[s]),
                "keys": np.ascontiguousarray(keys[s]),
                "values": np.ascontiguousarray(values[s]),
                "valid_lens": np.ascontiguousarray(valid_lens[s].reshape(1, BL)),
                "cblobA": cblobA,
                "cblobB": cblobB,
            }
        )
    return in_maps


def kernel(queries, keys, values, valid_lens, W_q, W_k, w_v):
    global _NC
    if _NC is None:
        _NC = build()
    in_maps = make_in_maps(queries, keys, values, valid_lens, W_q, W_k, w_v)
    res = run_bass_kernel_spmd(_NC, in_maps, core_ids=list(range(NCORES)))
    return np.concatenate([res.results[i]["out"] for i in range(NCORES)], axis=0)


# revision 36
# speedup vs baseline: 1.2370x; 1.1870x over previous
"""Additive (Bahdanau) attention on 8 TRN2 NeuronCores, data-parallel over batch.

Reference math (per batch b):
  qh = queries @ W_q            [Q, H]
  kh = keys @ W_k               [K, H]
  scores[q,k] = sum_h w_v[h] * tanh(qh[q,h] + kh[k,h])
  scores[q,k] = -1e6 where k >= valid_len[b]
  out = softmax_k(scores) @ values

Shapes: B=16, Q=64, K=1024, D=256, H=128. B is sharded 2 per core; no
collectives. The roofline is the ScalarE (ACT) tanh pass over B*Q*K*H
elements: 16.8M/core / 128 lanes / 1.2GHz ~= 110us; the kernel runs the
16 tanh instructions back-to-back and hides everything else under them
(measured ~120us ACT busy, ~145us total incl. ~8.5us NEFF startup and
~10us Tile drain barrier).

Device strategy per core (2 batches):
  - H=128 on the partition axis. keys/queries are PE-transposed (identity
    matmul) and projected with bf16 weights (pre-packed on host into one
    bf16 blob): khT [H, K] f32, qhT [H, Q] f32.
  - Per q: DVE tensor_scalar_add broadcasts qhT[:, q] over khT (fp32 2x
    mode, ~746ns); per q-group one big ACT Tanh -> bf16 features. Group
    schedule [4,4,8*6,4,2,2] shortens the first-tanh latency and the tail.
    Batch 0's first group fuses the add into ACT's per-partition bias
    (tanh(khT + qhT[:,q])) so the first tanh needs no DVE adds at all.
  - Per (q, k-chunk): matmul lhsT=features[H,128] (stationary), rhs=w_v
    [H,1] -> scoresT column [128k, 1] into a one-bank PSUM tile [128, 8*64].
  - Masking fuses into the Exp bias: madd[p,kc] = (p+128*kc >= len)*-1e6
    built from a constant iota input and a ones-matmul broadcast of
    valid_len. exp(score-1e6) underflows to exactly 0; scores are bounded
    (|s| <~ 12) so no max-subtraction is needed (softmax is shift-invariant).
  - attnT @ [values | ones] accumulates [Q, 257] over k-chunks; the ones
    column is the softmax denominator; one reciprocal + per-partition
    scale normalizes. values are cast f32->bf16 inside gpsimd SWDGE DMAs.
  - DMA: small-constants blob first, keys split across the sync HWDGE and
    gpsimd SWDGE queues, weights blob + valid_lens behind the first keys
    half, output on sync.
"""

import numpy as np

import concourse.bass as bass
import concourse.bacc as bacc
import concourse.mybir as mybir
import concourse.tile as tile
from concourse.bass_utils import run_bass_kernel_spmd

B, Q, K, D, H = 16, 64, 1024, 256, 128
NCORES = 8
BL = B // NCORES  # batches per core
KC = K // 128     # k-chunks of 128
DC = D // 128     # d-chunks of 128
QG = 8            # q-group size per Tanh instruction
NEG = -1.0e6

F32 = mybir.dt.float32
BF16 = mybir.dt.bfloat16
I32 = mybir.dt.int32
AF = mybir.ActivationFunctionType
ALU = mybir.AluOpType


def _emit(nc, tc, dram):
    queries, keys, values, vlens, cblobA, cblobB, out = dram
    QSCHED = [4, 4] + [8] * 6 + [4, 2, 2]
    assert sum(QSCHED) == Q
    with (
        tc.tile_pool(name="const", bufs=1) as cpool,
        tc.tile_pool(name="io", bufs=2) as io,
        tc.tile_pool(name="work", bufs=2) as work,
        tc.tile_pool(name="sums", bufs=3) as sums_pool,
        tc.tile_pool(name="feat", bufs=3) as feat_pool,
        tc.tile_pool(name="psT", bufs=3, space=bass.MemorySpace.PSUM) as psT,
        tc.tile_pool(name="psP", bufs=2, space=bass.MemorySpace.PSUM) as psP,
        tc.tile_pool(name="psS", bufs=2, space=bass.MemorySpace.PSUM) as psS,
        tc.tile_pool(name="psO", bufs=1, space=bass.MemorySpace.PSUM) as psO,
    ):
        cbA = cpool.tile([128, 265], F32, tag="cbA")
        cbB = cpool.tile([128, 513], BF16, tag="cbB")
        nc.sync.dma_start(cbA[:], cblobA[:, :])
        ident_sb = cbA[:, 0:128]
        ones_sb = cbA[0:1, 128:256]
        kidx_sb = cbA[:, 256:264]
        wq_bf = cbB[:, 0:256]
        wk_bf = cbB[:, 256:512]
        wv_bf = cbB[:, 512:513]
        vl_i = cpool.tile([1, BL], I32, tag="vli")
        vl_f = cpool.tile([1, BL], F32, tag="vlf")

        for b in range(BL):
            knat_all = io.tile([128, KC * D], F32, tag="knat")
            for kc in range(KC // 2):
                nc.sync.dma_start(
                    knat_all[:, kc * D : (kc + 1) * D],
                    keys[b, kc * 128 : (kc + 1) * 128, :],
                )
            for kc in range(KC // 2, KC):
                nc.gpsimd.dma_start(
                    knat_all[:, kc * D : (kc + 1) * D],
                    keys[b, kc * 128 : (kc + 1) * 128, :],
                )
            qnat = io.tile([Q, D], F32, tag="qnat")
            nc.sync.dma_start(qnat[:], queries[b, :, :])
            if b == 0:
                nc.sync.dma_start(cbB[:], cblobB[:, :])
                nc.sync.dma_start(vl_i[:], vlens[:, :])

            # ---- projections: khT [H, K] (half 0 first), qhT [H, Q] ----
            kTd = work.tile([128, DC * K], BF16, tag="kTd")
            khT = work.tile([128, K], F32, tag="khT")
            qT_sb = work.tile([128, DC * Q], BF16, tag="qT")
            qhT = work.tile([128, Q], F32, tag="qhT")

            def k_transposes(kc_list):
                for kc in kc_list:
                    for dc in range(DC):
                        tp = psT.tile([128, 128], F32, tag="tp")
                        nc.tensor.transpose(
                            tp[:],
                            knat_all[:, kc * D + dc * 128 : kc * D + (dc + 1) * 128],
                            ident_sb[:, :],
                        )
                        nc.vector.tensor_copy(
                            kTd[:, dc * K + kc * 128 : dc * K + (kc + 1) * 128], tp[:]
                        )

            def kh_half(nch):
                kh_ps = psP.tile([128, 512], F32, tag="proj")
                for dc in range(DC):
                    nc.tensor.matmul(
                        kh_ps[:],
                        wk_bf[:, dc * 128 : (dc + 1) * 128],
                        kTd[:, dc * K + nch * 512 : dc * K + nch * 512 + 512],
                        start=(dc == 0),
                        stop=(dc == DC - 1),
                    )
                nc.vector.tensor_copy(khT[:, nch * 512 : (nch + 1) * 512], kh_ps[:])

            k_transposes(range(0, KC // 2))
            kh_half(0)
            for dc in range(DC):
                tp = psT.tile([128, 128], F32, tag="tp")
                nc.tensor.transpose(
                    tp[:, 0:Q], qnat[:, dc * 128 : (dc + 1) * 128], ident_sb[0:Q, 0:Q]
                )
                nc.vector.tensor_copy(qT_sb[:, dc * Q : (dc + 1) * Q], tp[:, 0:Q])
            qh_ps = psP.tile([128, 512], F32, tag="proj")
            for dc in range(DC):
                nc.tensor.matmul(
                    qh_ps[:, 0:Q],
                    wq_bf[:, dc * 128 : (dc + 1) * 128],
                    qT_sb[:, dc * Q : (dc + 1) * Q],
                    start=(dc == 0),
                    stop=(dc == DC - 1),
                )
            nc.vector.tensor_copy(qhT[:], qh_ps[:, 0:Q])
            k_transposes(range(KC // 2, KC))
            kh_half(1)

            # ---- mask bias column: madd[p, kc] = (p + 128*kc >= len) * -1e6 ----
            if b == 0:
                nc.vector.tensor_copy(vl_f[:], vl_i[:])
            ln_ps = psT.tile([128, 128], F32, tag="tp")
            nc.tensor.matmul(
                ln_ps[:, 0:1], ones_sb, vl_f[0:1, b : b + 1], start=True, stop=True
            )
            ln_col = work.tile([128, 1], F32, tag="lncol")
            nc.vector.tensor_copy(ln_col[:], ln_ps[:, 0:1])
            madd = work.tile([128, KC], F32, tag="madd")
            nc.vector.tensor_scalar(
                madd[:], kidx_sb, ln_col[:], NEG, op0=ALU.is_ge, op1=ALU.mult
            )

            # ---- features + scoresT ----
            scT_ps = psS.tile([128, 512], F32, tag="sc")
            q0 = 0
            for g, qg in enumerate(QSCHED):
                feat = feat_pool.tile([128, qg * K], BF16, tag="feat")
                if b == 0 and g == 0:
                    for j in range(qg):
                        q = q0 + j
                        nc.scalar.activation(
                            feat[:, j * K : (j + 1) * K],
                            khT[:],
                            AF.Tanh,
                            bias=qhT[:, q : q + 1],
                        )
                else:
                    sums = sums_pool.tile([128, qg * K], F32, tag="sums")
                    for j in range(qg):
                        q = q0 + j
                        nc.vector.tensor_scalar_add(
                            sums[:, j * K : (j + 1) * K], khT[:], qhT[:, q : q + 1]
                        )
                    nc.scalar.activation(feat[:], sums[:], AF.Tanh)
                for j in range(qg):
                    q = q0 + j
                    for kc in range(KC):
                        nc.tensor.matmul(
                            scT_ps[:, kc * 64 + q : kc * 64 + q + 1],
                            feat[:, j * K + kc * 128 : j * K + (kc + 1) * 128],
                            wv_bf,
                            start=True,
                            stop=True,
                        )
                q0 += qg

            # ---- masked exp (bias fuses the mask) ----
            pT = work.tile([128, 512], BF16, tag="pT")
            for kc in range(KC):
                nc.scalar.activation(
                    pT[:, kc * 64 : (kc + 1) * 64],
                    scT_ps[:, kc * 64 : (kc + 1) * 64],
                    AF.Exp,
                    bias=madd[:, kc : kc + 1],
                )

            # ---- values (cast to bf16 in the SWDGE DMA) with ones columns ----
            vaug = work.tile([128, KC * 260], BF16, tag="vaug")
            for kc in range(KC):
                nc.gpsimd.dma_start(
                    vaug[:, kc * 260 : kc * 260 + 256],
                    values[b, kc * 128 : (kc + 1) * 128, :],
                )
                nc.gpsimd.memset(vaug[:, kc * 260 + 256 : kc * 260 + 257], 1.0)

            # ---- attnT @ [values | ones], normalize, store ----
            oaug_ps = psO.tile([Q, 257], F32, tag="oa")
            for kc in range(KC):
                nc.tensor.matmul(
                    oaug_ps[:],
                    pT[:, kc * 64 : (kc + 1) * 64],
                    vaug[:, kc * 260 : kc * 260 + 257],
                    start=(kc == 0),
                    stop=(kc == KC - 1),
                )
            recip = work.tile([Q, 1], F32, tag="recip")
            nc.vector.reciprocal(recip[:], oaug_ps[:, 256:257])
            out_sb = work.tile([Q, D], F32, tag="osb")
            nc.vector.tensor_scalar_mul(out_sb[:], oaug_ps[:, 0:256], recip[:])
            nc.sync.dma_start(out[b, :, :], out_sb[:])


def build():
    nc = bacc.Bacc("TRN2", target_bir_lowering=False, debug=False, num_devices=NCORES)
    dram = (
        nc.declare_dram_parameter("queries", [BL, Q, D], F32, isOutput=False),
        nc.declare_dram_parameter("keys", [BL, K, D], F32, isOutput=False),
        nc.declare_dram_parameter("values", [BL, K, D], F32, isOutput=False),
        nc.declare_dram_parameter("valid_lens", [1, BL], I32, isOutput=False),
        nc.declare_dram_parameter("cblobA", [128, 265], F32, isOutput=False),
        nc.declare_dram_parameter("cblobB", [128, 513], BF16, isOutput=False),
        nc.declare_dram_parameter("out", [BL, Q, D], F32, isOutput=True),
    )
    with tile.TileContext(nc) as tc:
        _emit(nc, tc, dram)
    nc.compile()
    return nc


_NC = None


def make_in_maps(queries, keys, values, valid_lens, W_q, W_k, w_v):
    queries = np.ascontiguousarray(np.asarray(queries, dtype=np.float32))
    keys = np.ascontiguousarray(np.asarray(keys, dtype=np.float32))
    values = np.ascontiguousarray(np.asarray(values, dtype=np.float32))
    valid_lens = np.asarray(valid_lens, dtype=np.int32)
    W_q = np.asarray(W_q, dtype=np.float32)
    W_k = np.asarray(W_k, dtype=np.float32)
    w_v = np.asarray(w_v, dtype=np.float32).reshape(H)
    cblobA = np.zeros((128, 265), dtype=np.float32)
    cblobA[:, 0:128] = np.eye(128, dtype=np.float32)
    cblobA[0, 128:256] = 1.0
    cblobA[:, 256:264] = (
        np.arange(128, dtype=np.float32)[:, None]
        + 128.0 * np.arange(KC, dtype=np.float32)[None, :]
    )
    cblobA[:, 264] = w_v
    import ml_dtypes
    cblobB = np.zeros((128, 513), dtype=ml_dtypes.bfloat16)
    cblobB[:, 0:128] = W_q[0:128, :].astype(ml_dtypes.bfloat16)
    cblobB[:, 128:256] = W_q[128:256, :].astype(ml_dtypes.bfloat16)
    cblobB[:, 256:384] = W_k[0:128, :].astype(ml_dtypes.bfloat16)
    cblobB[:, 384:512] = W_k[128:256, :].astype(ml_dtypes.bfloat16)
    cblobB[:, 512] = w_v.astype(ml_dtypes.bfloat16)
    in_maps = []
    for i in range(NCORES):
        s = slice(i * BL, (i + 1) * BL)
        in_maps.append(
            {
                "queries": np.ascontiguousarray(queries[s]),
                "keys": np.ascontiguousarray(keys[s]),
                "values": np.ascontiguousarray(values[s]),
                "valid_lens": np.ascontiguousarray(valid_lens[s].reshape(1, BL)),
                "cblobA": cblobA,
                "cblobB": cblobB,
            }
        )
    return in_maps


def kernel(queries, keys, values, valid_lens, W_q, W_k, w_v):
    global _NC
    if _NC is None:
        _NC = build()
    in_maps = make_in_maps(queries, keys, values, valid_lens, W_q, W_k, w_v)
    res = run_bass_kernel_spmd(_NC, in_maps, core_ids=list(range(NCORES)))
    return np.concatenate([res.results[i]["out"] for i in range(NCORES)], axis=0)
